# revision 2
# baseline (speedup 1.0000x reference)
"""Canny filter (blur -> sobel -> orientation-quantized NMS) on 8 Trainium2 cores.

v2: batch 16 sharded 2 images/core. Per core, 4 ALIGNED 128-row tiles (no
overlap): the vertical conv extent is handled by exact composed band matrices
(S_v . G_v with clamped edge folds) plus tiny cross-tile halo matmuls reading
the adjacent tile's z rows. All matmuls run as float32r (4x PE throughput).
Element-wise NMS work is split across DVE / Pool(gpsimd) / ACT engines.
"""
import sys
import numpy as np

sys.path.insert(0, "/opt/trn_rl_repo")

import concourse.bacc as bacc
import concourse.tile as tile
from concourse import mybir
from concourse.bass_utils import run_bass_kernel_spmd
from contextlib import ExitStack
from concourse.dve_ops import TENSOR_MASK

F32 = mybir.dt.float32
F32R = mybir.dt.float32r
U8 = mybir.dt.uint8

B, C, H, W = 16, 3, 512, 512
N_CORES = 8
B_PER = B // N_CORES          # 2 images per core
P = 128                       # partitions per tile
NT = 4                        # aligned row tiles per image
WP = W + 2                    # padded width per image
FW = B_PER * W                # 1024
FWP = B_PER * WP              # 1028

USE_F32R = True

_ALU = mybir.AluOpType
_ACTF = mybir.ActivationFunctionType

# angle-band thresholds: tan((2j-1)*pi/16)^2, j=1..4
_TJ2 = [float(np.tan((2 * j - 1) * np.pi / 16.0) ** 2) for j in (1, 2, 3, 4)]

_NC_CACHE = None


def _vertical_mats():
    """M[m, k] = weight of channel-summed row k in gx/gy row m (band +-2).
    Exact composition of (sobel vertical, blurred-level replication pad)
    after (gauss vertical, image-level replication pad)."""
    HH = NT * P
    v = np.array([np.exp(-0.5), 1.0, np.exp(-0.5)], np.float64)
    g1 = v / v.sum()
    G = np.zeros((HH, HH))
    for m in range(HH):
        for d, w in ((-1, g1[0]), (0, g1[1]), (1, g1[2])):
            G[m, min(max(m + d, 0), HH - 1)] += w
    MS = np.zeros((HH, HH))
    MD = np.zeros((HH, HH))
    for m in range(HH):
        for d, ws_, wd_ in ((-1, 1.0, -1.0), (0, 2.0, 0.0), (1, 1.0, 1.0)):
            k = min(max(m + d, 0), HH - 1)
            MS[m] += ws_ * G[k]
            MD[m] += wd_ * G[k]
    return MS, MD


def _build_weights():
    v = np.array([np.exp(-0.5), 1.0, np.exp(-0.5)], np.float64)
    sv = v.sum()
    h = v / (3.0 * sv)            # horizontal gauss taps (folds the /C)
    MS, MD = _vertical_mats()
    ws = {
        "Ih0": (np.eye(P) * h[0]).astype(np.float32),
        "Ih1": (np.eye(P) * h[1]).astype(np.float32),
    }
    # main bands: top(t=0), mid(t=1,2 identical), bottom(t=3)
    for tag, t in (("T", 0), ("M", 1), ("B", NT - 1)):
        lo = t * P
        S = MS[lo:lo + P, lo:lo + P].T
        D = MD[lo:lo + P, lo:lo + P].T
        ws["S_" + tag] = S.astype(np.float32)
        ws["Sn_" + tag] = (-S).astype(np.float32)
        ws["D_" + tag] = D.astype(np.float32)
        ws["D2_" + tag] = (2.0 * D).astype(np.float32)
    assert np.allclose(MS[P:2 * P, P:2 * P], MS[2 * P:3 * P, 2 * P:3 * P])
    # halo blocks (identical at every interior boundary)
    Su = MS[P:2 * P, 0:P].T       # out rows 0,1 read prev tile rows 126,127
    Du = MD[P:2 * P, 0:P].T
    Sd = MS[0:P, P:2 * P].T       # out rows 126,127 read next tile rows 0,1
    Dd = MD[0:P, P:2 * P].T
    ws["S_u"] = Su.astype(np.float32)
    ws["Sn_u"] = (-Su).astype(np.float32)
    ws["D_u"] = Du.astype(np.float32)
    ws["D2_u"] = (2.0 * Du).astype(np.float32)
    ws["S_d"] = Sd.astype(np.float32)
    ws["Sn_d"] = (-Sd).astype(np.float32)
    ws["D_d"] = Dd.astype(np.float32)
    ws["D2_d"] = (2.0 * Dd).astype(np.float32)
    return ws


def _build_program():
    nc = bacc.Bacc("TRN2", target_bir_lowering=False, debug=False, num_devices=N_CORES)
    img = nc.declare_dram_parameter("img", [B_PER, C, H, W], F32, isOutput=False)
    out = nc.declare_dram_parameter("out", [B_PER, 1, H, W], F32, isOutput=True)

    wnp = _build_weights()
    early_keys = ["Ih0", "Ih1"]
    rest_keys = sorted(k for k in wnp.keys() if k not in early_keys)
    wkeys = early_keys + rest_keys
    wdram_a = nc.inline_tensor(
        np.concatenate([wnp[k] for k in early_keys], axis=1), name="w_early")
    wdram_b = nc.inline_tensor(
        np.concatenate([wnp[k] for k in rest_keys], axis=1), name="w_rest")

    with tile.TileContext(nc) as tc, ExitStack() as ctx:
        cpool = ctx.enter_context(tc.tile_pool(name="consts", bufs=1))
        ctpool = ctx.enter_context(tc.tile_pool(name="cts", bufs=3))
        wpool = ctx.enter_context(tc.tile_pool(name="work", bufs=1))
        opool = ctx.enter_context(tc.tile_pool(name="osbp", bufs=2))
        mpool = ctx.enter_context(tc.tile_pool(name="masks", bufs=2))
        zpp = ctx.enter_context(tc.tile_pool(name="zps", bufs=2, space="PSUM"))
        gxpp = ctx.enter_context(tc.tile_pool(name="gxps", bufs=1, space="PSUM"))
        gypp = ctx.enter_context(tc.tile_pool(name="gyps", bufs=1, space="PSUM"))

        wall = cpool.tile([P, len(wkeys) * P], F32, tag="w_all")
        nE = len(early_keys)
        nc.sync.dma_start(wall[:, 0:nE * P].bitcast(F32R), wdram_a[:].bitcast(F32R))
        if USE_F32R:
            wsb = {k: wall[:, j * P:(j + 1) * P].bitcast(F32R)
                   for j, k in enumerate(wkeys)}
        else:
            wsb = {k: wall[:, j * P:(j + 1) * P] for j, k in enumerate(wkeys)}

        def mmr(psd, wkey, rhs, start=True, stop=True):
            r = rhs.bitcast(F32R) if USE_F32R else rhs
            nc.tensor.matmul(psd, wsb[wkey], r, start=start, stop=stop)

        zero = cpool.tile([1, FWP], F32, tag="zero")
        nc.vector.memset(zero[:], 0.0)

        zsb = [cpool.tile([P, FWP], F32, tag=f"z{t}", name=f"z{t}") for t in range(NT)]
        msqs = [cpool.tile([P, FWP], F32, tag=f"msq{t}", name=f"msq{t}") for t in range(NT)]
        for t in range(NT):
            mv = msqs[t][:, :].rearrange("p (i w) -> p i w", i=B_PER)
            nc.vector.memset(mv[:, :, 0:WP:WP - 1], 0.0)   # pad cols 0,513/image
        nsbs = [cpool.tile([P, FWP], F32, tag=f"nsb{j}", name=f"nsb{j}") for j in range(2)]
        ssbs = [cpool.tile([P, FWP], F32, tag=f"ssb{j}", name=f"ssb{j}") for j in range(2)]

        def im3(t):
            return t[:, :].rearrange("p (i w) -> p i w", i=B_PER)

        def shifted(base, off):
            return im3(base)[:, :, off:off + W]

        def stage_load(t):
            R = t * P
            c = ctpool.tile([P, FWP], F32, tag="ct")
            cv = im3(c)
            nc.sync.dma_start(
                cv[:, :, 1:1 + W].bitcast(F32R),
                img[:, 0, R:R + P, :].rearrange("i p w -> p i w").bitcast(F32R))
            for k in (1, 2):
                nc.gpsimd.dma_start(
                    cv[:, :, 1:1 + W].bitcast(F32R),
                    img[:, k, R:R + P, :].rearrange("i p w -> p i w").bitcast(F32R),
                    accum_op=_ALU.add)
            # replicate-pad cols 0,513 <- cols 1,512 (after channel accum)
            nc.scalar.copy(cv[:, :, 0:WP:WP - 1].bitcast(F32R),
                           cv[:, :, 1:WP:W - 1].bitcast(F32R))
            return c

        def stage_z(t, c):
            ps = zpp.tile([P, FW], F32, tag="z")
            for i in range(B_PER):
                o = i * WP
                s_ = slice(i * W, (i + 1) * W)
                mmr(ps[:, s_], "Ih0", c[:, o:o + W], start=True, stop=False)
                mmr(ps[:, s_], "Ih1", c[:, o + 1:o + 1 + W], start=False, stop=False)
                mmr(ps[:, s_], "Ih0", c[:, o + 2:o + 2 + W], start=False, stop=True)
            z = zsb[t]
            nc.scalar.copy(shifted(z, 1).bitcast(F32R), im3(ps))
            nc.scalar.copy(im3(z)[:, :, 0:WP:WP - 1].bitcast(F32R),
                           im3(z)[:, :, 1:WP:W - 1].bitcast(F32R))

        def stage_sobel(t):
            v = "T" if t == 0 else ("B" if t == NT - 1 else "M")
            z = zsb[t]
            ps_gx = gxpp.tile([P, FW], F32, tag="gx")
            ps_gy = gypp.tile([P, FW], F32, tag="gy")
            for i in range(B_PER):
                o = i * WP
                s_ = slice(i * W, (i + 1) * W)
                mmr(ps_gx[:, s_], "Sn_" + v, z[:, o:o + W], start=True, stop=False)
                mmr(ps_gx[:, s_], "S_" + v, z[:, o + 2:o + 2 + W],
                    start=False, stop=(t == 0 and NT == 1))
                if t > 0:
                    zp = zsb[t - 1]
                    mmr(ps_gx[:, s_], "Sn_u", zp[:, o:o + W], start=False, stop=False)
                    mmr(ps_gx[:, s_], "S_u", zp[:, o + 2:o + 2 + W],
                        start=False, stop=(t == NT - 1))
                if t < NT - 1:
                    zn = zsb[t + 1]
                    mmr(ps_gx[:, s_], "Sn_d", zn[:, o:o + W], start=False, stop=False)
                    mmr(ps_gx[:, s_], "S_d", zn[:, o + 2:o + 2 + W],
                        start=False, stop=True)

                mmr(ps_gy[:, s_], "D_" + v, z[:, o:o + W], start=True, stop=False)
                mmr(ps_gy[:, s_], "D2_" + v, z[:, o + 1:o + 1 + W],
                    start=False, stop=False)
                mmr(ps_gy[:, s_], "D_" + v, z[:, o + 2:o + 2 + W],
                    start=False, stop=(t == 0 and NT == 1))
                if t > 0:
                    zp = zsb[t - 1]
                    mmr(ps_gy[:, s_], "D_u", zp[:, o:o + W], start=False, stop=False)
                    mmr(ps_gy[:, s_], "D2_u", zp[:, o + 1:o + 1 + W],
                        start=False, stop=False)
                    mmr(ps_gy[:, s_], "D_u", zp[:, o + 2:o + 2 + W],
                        start=False, stop=(t == NT - 1))
                if t < NT - 1:
                    zn = zsb[t + 1]
                    mmr(ps_gy[:, s_], "D_d", zn[:, o:o + W], start=False, stop=False)
                    mmr(ps_gy[:, s_], "D2_d", zn[:, o + 1:o + 1 + W],
                        start=False, stop=False)
                    mmr(ps_gy[:, s_], "D_d", zn[:, o + 2:o + 2 + W],
                        start=False, stop=True)
            return ps_gx, ps_gy

        def stage_masks(t, ps_gx, ps_gy):
            sqx = wpool.tile([P, FW], F32, tag="sqx")
            nc.scalar.activation(sqx[:], ps_gx[:], _ACTF.Square)
            sqy = wpool.tile([P, FW], F32, tag="sqy")
            nc.scalar.activation(sqy[:], ps_gy[:], _ACTF.Square)
            gx_sb = wpool.tile([P, FW], F32, tag="gx_sb")
            nc.scalar.copy(gx_sb[:], ps_gx[:])
            gy_sb = wpool.tile([P, FW], F32, tag="gy_sb")
            nc.scalar.copy(gy_sb[:], ps_gy[:])

            mc = shifted(msqs[t], 1)
            nc.gpsimd.tensor_tensor(
                mc, sqx[:].rearrange("p (i w) -> p i w", i=B_PER),
                sqy[:].rearrange("p (i w) -> p i w", i=B_PER), _ALU.add)
            osb = opool.tile([P, FW], F32, tag="osb")
            nc.scalar.activation(im3(osb), mc, _ACTF.Sqrt, scale=0.25)

            rx = wpool.tile([P, FW], F32, tag="rx")
            nc.vector.reciprocal_approx_fast(rx[:], sqx[:])
            trat = wpool.tile([P, FW], F32, tag="trat")
            nc.gpsimd.tensor_tensor(trat[:], sqy[:], rx[:], _ALU.mult)
            gxy = wpool.tile([P, FW], F32, tag="gxy")
            nc.gpsimd.tensor_tensor(gxy[:], gx_sb[:], gy_sb[:], _ALU.mult)

            cms = []
            for j, tj2 in enumerate(_TJ2):
                cm = mpool.tile([P, FW], U8, tag=f"c{j}m")
                nc.vector.tensor_scalar(cm[:], trat[:], tj2, None, _ALU.is_gt)
                cms.append(cm)
            q = mpool.tile([P, FW], U8, tag="q")
            nc.vector.scalar_tensor_tensor(q[:], gxy[:], 0.0, cms[2][:],
                                           _ALU.is_gt, _ALU.not_equal)
            return dict(cms=cms, q=q, osb=osb)

        def stage_shifts(t):
            nsb, ssb = nsbs[t % 2], ssbs[t % 2]
            nc.sync.dma_start(nsb[1:P, :], msqs[t][0:P - 1, :])
            if t > 0:
                nc.sync.dma_start(nsb[0:1, :], msqs[t - 1][P - 1:P, :])
            else:
                nc.sync.dma_start(nsb[0:1, :], zero[0:1, :])
            nc.scalar.dma_start(ssb[0:P - 1, :], msqs[t][1:P, :])
            if t < NT - 1:
                nc.scalar.dma_start(ssb[P - 1:P, :], msqs[t + 1][0:1, :])
            else:
                nc.scalar.dma_start(ssb[P - 1:P, :], zero[0:1, :])

        def stage_b(t, st):
            nsb, ssb = nsbs[t % 2], ssbs[t % 2]
            cms, q, osb = st["cms"], st["q"], st["osb"]
            msq = msqs[t]
            mc = shifted(msq, 1)

            a1 = wpool.tile([P, FW], F32, tag="a1")   # NE / SW
            nc.vector.tensor_tensor(im3(a1), shifted(nsb, 2), shifted(ssb, 0),
                                    _ALU.max)
            a3 = wpool.tile([P, FW], F32, tag="a3")   # NW / SE
            nc.vector.tensor_tensor(im3(a3), shifted(nsb, 0), shifted(ssb, 2),
                                    _ALU.max)
            # a2 = max(N, S) decomposed as N + relu(S - N) (Pool + ACT)
            d2 = wpool.tile([P, FW], F32, tag="d2")
            nc.gpsimd.tensor_tensor(im3(d2), shifted(ssb, 1), shifted(nsb, 1),
                                    _ALU.subtract)
            r2 = wpool.tile([P, FW], F32, tag="r2")
            nc.scalar.activation(r2[:], d2[:], _ACTF.Relu)
            a2 = wpool.tile([P, FW], F32, tag="a2")
            nc.gpsimd.tensor_tensor(im3(a2), im3(r2), shifted(nsb, 1), _ALU.add)

            m = wpool.tile([P, FW], F32, tag="m")     # E / W
            nc.vector.tensor_tensor(im3(m), shifted(msq, 0), shifted(msq, 2),
                                    _ALU.max)
            a0c = wpool.tile([P, FW], F32, tag="a0c")
            nc.scalar.copy(a0c[:], m[:])
            nc.vector.copy_predicated(a3[:], q[:], a1[:])   # a3 -> adiag
            nc.vector.copy_predicated(m[:], cms[0][:], a3[:])
            nc.vector.copy_predicated(m[:], cms[1][:], a2[:])
            nc.vector.copy_predicated(m[:], cms[2][:], a3[:])
            nc.vector.copy_predicated(m[:], cms[3][:], a0c[:])

            dsub = wpool.tile([P, FW], F32, tag="dsub")
            nc.gpsimd.tensor_tensor(im3(dsub), im3(m), mc, _ALU.subtract)
            thin = wpool.tile([P, FW], F32, tag="thin")
            nc.vector._custom_dve(TENSOR_MASK, out=thin[:], in0=osb[:],
                                  in1=dsub[:], s0=0.0, s1=0.0, imm2=0.0)

            R = t * P
            nc.sync.dma_start(
                out[:, 0, R:R + P, :].rearrange("i r w -> r i w"), im3(thin))

        # software-pipelined emission:
        #   load(k), z(k-1), sobel/masks(k-2), shifts+stage_b(k-3)
        nc.scalar.dma_start(wall[:, nE * P:].bitcast(F32R), wdram_b[:].bitcast(F32R))
        cs = [None] * NT
        sts = [None] * NT
        for step in range(NT + 3):
            if step < NT:
                cs[step] = stage_load(step)
            if 1 <= step <= NT:
                t = step - 1
                stage_z(t, cs[t])
                cs[t] = None
            if 2 <= step <= NT + 1:
                t = step - 2
                pss = stage_sobel(t)
                sts[t] = stage_masks(t, *pss)
            if step >= 3:
                t = step - 3
                stage_shifts(t)
                stage_b(t, sts[t])
                sts[t] = None

    nc.compile()
    return nc


def _get_program():
    global _NC_CACHE
    if _NC_CACHE is None:
        _NC_CACHE = _build_program()
    return _NC_CACHE


def kernel(img, w_gauss=None, w_sobel_x=None, w_sobel_y=None, w_dir=None):
    img = np.ascontiguousarray(np.asarray(img, dtype=np.float32))
    assert img.shape == (B, C, H, W)
    nc = _get_program()
    in_maps = [{"img": img[c * B_PER:(c + 1) * B_PER]} for c in range(N_CORES)]
    res = run_bass_kernel_spmd(nc, in_maps, list(range(N_CORES)))
    return np.concatenate([res.results[c]["out"] for c in range(N_CORES)], axis=0)
